# revision 34
# baseline (speedup 1.0000x reference)
"""DMTetGeometry kernel for 8 Trainium2 NeuronCores.

Split of work:
  - device (8 cores, data-parallel over vertices): the 5-layer SDF MLP
    (27 -> 256 -> 256 -> 256 -> 256 -> 1), which is all of the FLOPs.
    Activations are kept feature-major ([features, rows]) so every layer
    is a plain K-on-partitions matmul with zero transposes.
  - host: positional encoding (sin/cos must match the fp32 reference
    closely; the ACT engine's table-based Sin with fp32 range reduction
    is not accurate enough for the sign-critical sdf), and the marching
    tetrahedra stage (data-dependent shapes: unique/cumsum/masking).

The output's discrete structure depends on sign(sdf), so the MLP is run
in full fp32 on the PE array.
"""

import sys

for _p in ("/root/.axon_site/_ro/trn_rl_repo", "/opt/trn_rl_repo"):
    if _p not in sys.path:
        sys.path.append(_p)

import numpy as np

import concourse.bacc as bacc
import concourse.mybir as mybir
import concourse.tile as tile
from concourse.bass_utils import run_bass_kernel_spmd

N_CORES = 8
N_VERTS = 200000
PER_CORE = 25000
BLK = 512
NBLK = (PER_CORE + BLK - 1) // BLK  # 49
PER_CORE_PAD = NBLK * BLK  # 25088
D_IN = 27
D_PAD = 27  # no K padding (measured: padding to 32 buys nothing on the PE)
HID = 256
FREQ_NUM = 4

_TRI_TABLE = np.array([
    [-1,-1,-1,-1,-1,-1],[1,0,2,-1,-1,-1],[4,0,3,-1,-1,-1],[1,4,2,1,3,4],
    [3,1,5,-1,-1,-1],[2,3,0,2,5,3],[1,4,0,1,5,4],[4,2,5,-1,-1,-1],
    [4,5,2,-1,-1,-1],[4,1,0,4,5,1],[3,2,0,3,5,2],[1,3,5,-1,-1,-1],
    [4,1,2,4,3,1],[3,0,4,-1,-1,-1],[2,0,1,-1,-1,-1],[-1,-1,-1,-1,-1,-1]], dtype=np.int32)
_NUM_TRI = np.array([0,1,1,2,1,2,2,1,1,2,2,1,2,1,1,0], dtype=np.int32)
_BASE_EDGES = np.array([0,1,0,2,0,3,1,2,1,3,2,3], dtype=np.int32)

F32 = mybir.dt.float32
F16 = mybir.dt.float16
F32R = mybir.dt.float32r
Relu = mybir.ActivationFunctionType.Relu
Identity = mybir.ActivationFunctionType.Identity
USE_F32R = False


def _mm(ap):
    return ap.bitcast(F32R) if USE_F32R else ap


def _build_nc(nblk=NBLK):
    n_cols = nblk * BLK
    nc = bacc.Bacc("TRN2", target_bir_lowering=False, debug=False,
                   enable_asserts=False)
    xa = nc.dram_tensor("xa", [D_PAD, n_cols], F16, kind="ExternalInput")
    xb = nc.dram_tensor("xb", [D_PAD, n_cols], F16, kind="ExternalInput")
    w0a = nc.dram_tensor("w0a", [D_PAD, HID], F16, kind="ExternalInput")
    w0b = nc.dram_tensor("w0b", [D_PAD, HID], F16, kind="ExternalInput")
    # hidden weights pre-chunked on host: [k_chunk, 128, 256]
    wh = [nc.dram_tensor(f"w{l}c", [2, 128, HID], F32, kind="ExternalInput")
          for l in (1, 2, 3)]
    # biases pre-chunked on host: [128, 2]
    bs = [nc.dram_tensor(f"b{l}c", [128, 2], F32, kind="ExternalInput")
          for l in (0, 1, 2, 3)]
    wf = nc.dram_tensor("wfc", [128, 2], F32, kind="ExternalInput")
    bf = nc.dram_tensor("bfc", [1, 1], F32, kind="ExternalInput")
    sdf = nc.dram_tensor("sdf", [1, n_cols], F32, kind="ExternalOutput")

    with tile.TileContext(nc) as tc:
        with (
            tc.tile_pool(name="consts", bufs=1) as cpool,
            tc.tile_pool(name="xin", bufs=4) as xpool,
            tc.tile_pool(name="acts", bufs=22) as hpool,
            tc.tile_pool(name="souts", bufs=4) as spool,
            tc.tile_pool(name="ps", bufs=6, space="PSUM") as ppool,
            tc.tile_pool(name="psf", bufs=2, space="PSUM") as pfpool,
        ):
            # HAM warm-up + first-pair input + layer-0 consts go FIRST so
            # the PE has real work within ~2 us; remaining weights stream in
            # behind them.
            warm = cpool.tile([128, 128], F32, tag="warm")
            nc.gpsimd.memset(warm[:], 0.0)
            wps = pfpool.tile([128, 64], F32, tag="psf", name="wps")
            for _ in range(10):
                nc.tensor.matmul(wps[:], lhsT=_mm(warm[:]),
                                 rhs=_mm(warm[:, :64]), start=True, stop=True)

            xta0 = xpool.tile([D_PAD, 2 * BLK], F16, tag="xta")
            nc.sync.dma_start(out=xta0[:], in_=xa[:, 0:2 * BLK])
            xtb0 = xpool.tile([D_PAD, 2 * BLK], F16, tag="xtb")
            nc.sync.dma_start(out=xtb0[:], in_=xb[:, 0:2 * BLK])
            w0a_sb = cpool.tile([D_PAD, HID], F16, tag="w0a")
            nc.sync.dma_start(out=w0a_sb[:], in_=w0a[:, :])
            w0b_sb = cpool.tile([D_PAD, HID], F16, tag="w0b")
            nc.sync.dma_start(out=w0b_sb[:], in_=w0b[:, :])
            wh_sb = []
            for li, w in enumerate(wh):
                pair = []
                for k in range(2):
                    t = cpool.tile([128, HID], F32, tag=f"w{li}{k}")
                    nc.sync.dma_start(out=t[:], in_=w[k])
                    pair.append(t)
                wh_sb.append(pair)
            bs_sb = []
            for li, b in enumerate(bs):
                t = cpool.tile([128, 2], F32, tag=f"b{li}")
                nc.sync.dma_start(out=t[:], in_=b[:, :])
                bs_sb.append(t)
            wf_sb = cpool.tile([128, 2], F32, tag="wf")
            nc.sync.dma_start(out=wf_sb[:], in_=wf[:, :])
            bf_sb = cpool.tile([1, 1], F32, tag="bf")
            nc.sync.dma_start(out=bf_sb[:], in_=bf[:, :])
            ones_sb = cpool.tile([128, 1], F32, tag="ones")
            nc.gpsimd.memset(ones_sb[:], 1.0)

            def bias_relu(h, ps, b_ap, use_act):
                if use_act:
                    nc.scalar.activation(h[:], ps[:], Relu, bias=b_ap)
                else:
                    nc.vector.tensor_scalar(
                        h[:], ps[:], b_ap, 0.0,
                        mybir.AluOpType.add, mybir.AluOpType.max)

            # process blocks in pairs so consecutive matmuls share the
            # same stationary (lhsT) operand
            for i0 in range(0, nblk, 2):
                blks = [i0] if i0 + 1 >= nblk else [i0, i0 + 1]
                nb = len(blks)
                cols = [slice(i * BLK, (i + 1) * BLK) for i in blks]
                if i0 == 0:
                    xta, xtb = xta0, xtb0
                else:
                    xta = xpool.tile([D_PAD, nb * BLK], F16, tag="xta")
                    nc.sync.dma_start(
                        out=xta[:], in_=xa[:, i0 * BLK:(i0 + nb) * BLK])
                    xtb = xpool.tile([D_PAD, nb * BLK], F16, tag="xtb")
                    nc.sync.dma_start(
                        out=xtb[:], in_=xb[:, i0 * BLK:(i0 + nb) * BLK])
                xtas = [xta[:, b * BLK:(b + 1) * BLK] for b in range(nb)]
                xtbs = [xtb[:, b * BLK:(b + 1) * BLK] for b in range(nb)]

                # layer 0: 27 -> 256, exact fp16x2 split
                # (x = x1 + x2/2048, w0 = w1 + w2/2048; the x2*w2 term is
                # ~2^-22 relative and dropped; fp16 products are exact in
                # the fp32 PSUM accumulator)
                psa = {}
                psb = {}
                for m in range(2):
                    ms = slice(m * 128, (m + 1) * 128)
                    for b in range(nb):
                        pa = ppool.tile([128, BLK], F32, tag="ps", name="pa")
                        nc.tensor.matmul(pa[:], lhsT=w0a_sb[:, ms],
                                         rhs=xtas[b], start=True, stop=True)
                        psa[(b, m)] = pa
                    for b in range(nb):
                        pb = ppool.tile([128, BLK], F32, tag="ps", name="pb")
                        nc.tensor.matmul(pb[:], lhsT=w0a_sb[:, ms],
                                         rhs=xtbs[b], start=True, stop=False)
                        nc.tensor.matmul(pb[:], lhsT=w0b_sb[:, ms],
                                         rhs=xtas[b], start=False, stop=True)
                        psb[(b, m)] = pb
                hcur = [[None, None] for _ in range(nb)]
                for m in range(2):
                    for b in range(nb):
                        u = hpool.tile([128, BLK], F32, tag="h", name="u")
                        nc.scalar.activation(u[:], psb[(b, m)][:], Identity,
                                             bias=bs_sb[0][:, m:m + 1],
                                             scale=1.0 / 2048.0)
                        t = hpool.tile([128, BLK], F32, tag="h", name="t")
                        nc.vector.tensor_tensor(t[:], psa[(b, m)][:], u[:],
                                                mybir.AluOpType.add)
                        h = hpool.tile([128, BLK], F32, tag="h")
                        if m == 0:
                            nc.scalar.activation(h[:], t[:], Relu)
                        else:
                            nc.vector.tensor_scalar(h[:], t[:], 0.0, None,
                                                    mybir.AluOpType.max)
                        hcur[b][m] = h

                # layers 1..3: 256 -> 256
                for li in range(3):
                    pss = {}
                    for m in range(2):
                        for k in range(2):
                            for b in range(nb):
                                if k == 0:
                                    ps = ppool.tile([128, BLK], F32, tag="ps")
                                    pss[(b, m)] = ps
                                nc.tensor.matmul(
                                    pss[(b, m)][:],
                                    lhsT=_mm(wh_sb[li][k][:, m * 128:(m + 1) * 128]),
                                    rhs=_mm(hcur[b][k][:]),
                                    start=(k == 0), stop=(k == 1))
                    hnew = [[None, None] for _ in range(nb)]
                    for m in range(2):
                        for b in range(nb):
                            h = hpool.tile([128, BLK], F32, tag="h")
                            bias_relu(h, pss[(b, m)], bs_sb[li + 1][:, m:m + 1],
                                      use_act=(m == 0))
                            hnew[b][m] = h
                    hcur = hnew

                # final: 256 -> 1.  Fold wf into the activations
                # elementwise (per-partition scale) and reduce the 128
                # partitions with a constant ones matmul: one PE pass per
                # block instead of two.
                psfs = []
                for b in range(nb):
                    t1 = hpool.tile([128, BLK], F32, tag="zf", name="t1")
                    nc.scalar.activation(t1[:], hcur[b][0][:],
                                         mybir.ActivationFunctionType.Copy,
                                         scale=wf_sb[:, 0:1])
                    t2 = hpool.tile([128, BLK], F32, tag="zf", name="t2")
                    nc.vector.tensor_scalar(t2[:], hcur[b][1][:],
                                            wf_sb[:, 1:2], None,
                                            mybir.AluOpType.mult)
                    z = hpool.tile([128, BLK], F32, tag="zf", name="z")
                    nc.vector.tensor_tensor(z[:], t1[:], t2[:],
                                            mybir.AluOpType.add)
                    psf = pfpool.tile([1, BLK], F32, tag="psf", name="psf")
                    nc.tensor.matmul(psf[:], lhsT=_mm(ones_sb[:]),
                                     rhs=_mm(z[:]), start=True, stop=True)
                    psfs.append(psf)
                for b in range(nb):
                    so = spool.tile([1, BLK], F32, tag="so")
                    nc.scalar.activation(so[:], psfs[b][:], Identity,
                                         bias=bf_sb[0:1, 0:1])
                    nc.sync.dma_start(out=sdf[:, cols[b]], in_=so[:])

    nc.compile()
    return nc


_NC_CACHE = {}


def _get_nc():
    if "nc" not in _NC_CACHE:
        _NC_CACHE["nc"] = _build_nc()
    return _NC_CACHE["nc"]


def _posenc_t(pos):
    """Feature-major positional encoding [27, N], fp32, matching the
    reference's fp32 elementwise ops."""
    n = pos.shape[0]
    posT = np.ascontiguousarray(pos.T.astype(np.float32, copy=False))  # [3, N]
    x = np.zeros((D_PAD, n), np.float32)
    x[0:3] = posT
    for i in range(FREQ_NUM):
        f = np.float32(float(2 ** i) * np.pi)
        fx = f * posT
        x[3 + 6 * i: 6 + 6 * i] = np.sin(fx)
        x[6 + 6 * i: 9 + 6 * i] = np.cos(fx)
    return x


def _split16(a):
    """Exact two-term fp16 split: a ~= a1 + a2/2048 to ~22 mantissa bits.
    a - fp32(a1) is exact (operands within 2x), *2048 is exact."""
    a1 = a.astype(np.float16)
    a2 = ((a - a1.astype(np.float32)) * np.float32(2048.0)).astype(np.float16)
    return np.ascontiguousarray(a1), np.ascontiguousarray(a2)


def _chunk_inputs(w0, b0, w1, b1, w2, b2, w3, b3, wf, bf):
    f32 = lambda a: np.ascontiguousarray(np.asarray(a), dtype=np.float32)
    w0p = np.zeros((D_PAD, HID), np.float32)
    w0p[:D_IN] = f32(w0)
    w0a, w0b = _split16(w0p)
    out = {"w0a": w0a, "w0b": w0b}
    for name, w in (("w1c", w1), ("w2c", w2), ("w3c", w3)):
        w = f32(w)
        out[name] = np.ascontiguousarray(np.stack([w[:128], w[128:]]))
    for name, b in (("b0c", b0), ("b1c", b1), ("b2c", b2), ("b3c", b3)):
        b = f32(b)
        out[name] = np.ascontiguousarray(b.reshape(2, 128).T)
    wf = f32(wf).reshape(-1)
    out["wfc"] = np.ascontiguousarray(wf.reshape(2, 128).T)
    out["bfc"] = f32(bf).reshape(1, 1)
    return out


def _sdf_device(x_t, weight_maps):
    """x_t: [27, N_VERTS] fp32 -> sdf [N_VERTS] fp32 via 8-core SPMD."""
    nc = _get_nc()
    in_maps = []
    for c in range(N_CORES):
        xc = np.zeros((D_PAD, PER_CORE_PAD), np.float32)
        xc[:, :PER_CORE] = x_t[:, c * PER_CORE:(c + 1) * PER_CORE]
        xca, xcb = _split16(xc)
        m = {"xa": xca, "xb": xcb}
        m.update(weight_maps)
        in_maps.append(m)
    res = run_bass_kernel_spmd(nc, in_maps, list(range(N_CORES)))
    sdf = np.empty(N_VERTS, np.float32)
    for c in range(N_CORES):
        sdf[c * PER_CORE:(c + 1) * PER_CORE] = \
            res.results[c]["sdf"][0, :PER_CORE]
    return sdf


def _march_tets(pos, tet_fx4, sdf):
    """Marching tetrahedra, numpy mirror of the jnp reference."""
    occ = sdf > 0
    occ_fx4 = occ[tet_fx4]                       # [F,4]
    occ_sum = occ_fx4.sum(-1)
    valid = (occ_sum > 0) & (occ_sum < 4)
    vt = tet_fx4[valid]                          # [V,4]
    occ_v = occ_fx4[valid]

    # Only crossing edges (occ[lo] != occ[hi]) ever appear in the
    # triangle tables, and the reference's compact ids are the ranks of
    # the crossing edges in (lo, hi)-sorted order — so uniquify just the
    # crossing instances; ranks and interp vertices come out identical.
    e = vt[:, _BASE_EDGES].reshape(-1, 2)        # [6V,2]
    e0, e1 = e[:, 0], e[:, 1]
    ce = occ[e0] != occ[e1]                      # crossing instances
    lo = np.minimum(e0[ce], e1[ce]).astype(np.int64)
    hi = np.maximum(e0[ce], e1[ce]).astype(np.int64)
    key = lo * N_VERTS + hi
    uniq_key, inv = np.unique(key, return_inverse=True)
    idx_flat = np.full(e.shape[0], -1, np.int32)
    idx_flat[ce] = inv.astype(np.int32)
    idx_map = idx_flat.reshape(-1, 6)

    ilo = uniq_key // N_VERTS
    ihi = uniq_key % N_VERTS
    s0 = sdf[ilo]
    s1 = sdf[ihi]
    denom = (s0 - s1)[:, None]
    verts = pos[ilo]
    verts = verts * (-s1)[:, None]
    np.add(verts, pos[ihi] * s0[:, None], out=verts)
    np.divide(verts, denom, out=verts)

    tetindex = (occ_v.astype(np.int32) *
                np.array([1, 2, 4, 8], np.int32)).sum(-1)
    ntri = _NUM_TRI[tetindex]
    tri = _TRI_TABLE[tetindex]
    m1 = ntri == 1
    m2 = ntri == 2
    f1 = np.take_along_axis(idx_map[m1], tri[m1][:, :3], axis=1).reshape(-1, 3)
    f2 = np.take_along_axis(idx_map[m2], tri[m2][:, :6], axis=1).reshape(-1, 3)
    faces = np.concatenate([f1, f2], axis=0).astype(np.int32)
    return verts.astype(np.float32), faces


def kernel(pos, tet_fx4, w0, b0, w1, b1, w2, b2, w3, b3, wf, bf):
    pos = np.ascontiguousarray(np.asarray(pos), dtype=np.float32)
    tet_fx4 = np.ascontiguousarray(np.asarray(tet_fx4), dtype=np.int32)
    x_t = _posenc_t(pos)
    wm = _chunk_inputs(w0, b0, w1, b1, w2, b2, w3, b3, wf, bf)
    sdf = _sdf_device(x_t, wm)
    return _march_tets(pos, tet_fx4, sdf)


# revision 35
# speedup vs baseline: 1.0034x; 1.0034x over previous
"""DMTetGeometry kernel for 8 Trainium2 NeuronCores.

Split of work:
  - device (8 cores, data-parallel over vertices): the 5-layer SDF MLP
    (27 -> 256 -> 256 -> 256 -> 256 -> 1), which is all of the FLOPs.
    Activations are kept feature-major ([features, rows]) so every layer
    is a plain K-on-partitions matmul with zero transposes.
  - host: positional encoding (sin/cos must match the fp32 reference
    closely; the ACT engine's table-based Sin with fp32 range reduction
    is not accurate enough for the sign-critical sdf), and the marching
    tetrahedra stage (data-dependent shapes: unique/cumsum/masking).

The output's discrete structure depends on sign(sdf), so the MLP is run
in full fp32 on the PE array.
"""

import sys

for _p in ("/root/.axon_site/_ro/trn_rl_repo", "/opt/trn_rl_repo"):
    if _p not in sys.path:
        sys.path.append(_p)

import numpy as np

import concourse.bacc as bacc
import concourse.mybir as mybir
import concourse.tile as tile
from concourse.bass_utils import run_bass_kernel_spmd

N_CORES = 8
N_VERTS = 200000
PER_CORE = 25000
BLK = 512
NBLK = (PER_CORE + BLK - 1) // BLK  # 49
PER_CORE_PAD = NBLK * BLK  # 25088
D_IN = 27
D_PAD = 27  # no K padding (measured: padding to 32 buys nothing on the PE)
HID = 256
FREQ_NUM = 4

_TRI_TABLE = np.array([
    [-1,-1,-1,-1,-1,-1],[1,0,2,-1,-1,-1],[4,0,3,-1,-1,-1],[1,4,2,1,3,4],
    [3,1,5,-1,-1,-1],[2,3,0,2,5,3],[1,4,0,1,5,4],[4,2,5,-1,-1,-1],
    [4,5,2,-1,-1,-1],[4,1,0,4,5,1],[3,2,0,3,5,2],[1,3,5,-1,-1,-1],
    [4,1,2,4,3,1],[3,0,4,-1,-1,-1],[2,0,1,-1,-1,-1],[-1,-1,-1,-1,-1,-1]], dtype=np.int32)
_NUM_TRI = np.array([0,1,1,2,1,2,2,1,1,2,2,1,2,1,1,0], dtype=np.int32)
_BASE_EDGES = np.array([0,1,0,2,0,3,1,2,1,3,2,3], dtype=np.int32)

F32 = mybir.dt.float32
F16 = mybir.dt.float16
F32R = mybir.dt.float32r
Relu = mybir.ActivationFunctionType.Relu
Identity = mybir.ActivationFunctionType.Identity
USE_F32R = False


def _mm(ap):
    return ap.bitcast(F32R) if USE_F32R else ap


def _build_nc(nblk=NBLK):
    n_cols = nblk * BLK
    nc = bacc.Bacc("TRN2", target_bir_lowering=False, debug=False,
                   enable_asserts=False)
    xa = nc.dram_tensor("xa", [D_PAD, n_cols], F16, kind="ExternalInput")
    xb = nc.dram_tensor("xb", [D_PAD, n_cols], F16, kind="ExternalInput")
    w0a = nc.dram_tensor("w0a", [D_PAD, HID], F16, kind="ExternalInput")
    w0b = nc.dram_tensor("w0b", [D_PAD, HID], F16, kind="ExternalInput")
    # hidden weights pre-chunked on host: [k_chunk, 128, 256]
    wh = [nc.dram_tensor(f"w{l}c", [2, 128, HID], F32, kind="ExternalInput")
          for l in (1, 2, 3)]
    # biases pre-chunked on host: [128, 2]
    bs = [nc.dram_tensor(f"b{l}c", [128, 2], F32, kind="ExternalInput")
          for l in (0, 1, 2, 3)]
    wf = nc.dram_tensor("wfc", [128, 2], F32, kind="ExternalInput")
    bf = nc.dram_tensor("bfc", [1, 1], F32, kind="ExternalInput")
    sdf = nc.dram_tensor("sdf", [1, n_cols], F32, kind="ExternalOutput")

    with tile.TileContext(nc) as tc:
        with (
            tc.tile_pool(name="consts", bufs=1) as cpool,
            tc.tile_pool(name="xin", bufs=4) as xpool,
            tc.tile_pool(name="acts", bufs=16) as hpool,
            tc.tile_pool(name="souts", bufs=4) as spool,
            tc.tile_pool(name="ps", bufs=6, space="PSUM") as ppool,
            tc.tile_pool(name="psf", bufs=2, space="PSUM") as pfpool,
        ):
            # HAM warm-up + first-pair input + layer-0 consts go FIRST so
            # the PE has real work within ~2 us; remaining weights stream in
            # behind them.
            warm = cpool.tile([128, 128], F32, tag="warm")
            nc.gpsimd.memset(warm[:], 0.0)
            wps = pfpool.tile([128, 64], F32, tag="psf", name="wps")
            for _ in range(10):
                nc.tensor.matmul(wps[:], lhsT=_mm(warm[:]),
                                 rhs=_mm(warm[:, :64]), start=True, stop=True)

            xta0 = xpool.tile([D_PAD, 2 * BLK], F16, tag="xta")
            nc.sync.dma_start(out=xta0[:], in_=xa[:, 0:2 * BLK])
            xtb0 = xpool.tile([D_PAD, 2 * BLK], F16, tag="xtb")
            nc.sync.dma_start(out=xtb0[:], in_=xb[:, 0:2 * BLK])
            w0a_sb = cpool.tile([D_PAD, HID], F16, tag="w0a")
            nc.sync.dma_start(out=w0a_sb[:], in_=w0a[:, :])
            w0b_sb = cpool.tile([D_PAD, HID], F16, tag="w0b")
            nc.sync.dma_start(out=w0b_sb[:], in_=w0b[:, :])
            wh_sb = []
            for li, w in enumerate(wh):
                pair = []
                for k in range(2):
                    t = cpool.tile([128, HID], F32, tag=f"w{li}{k}")
                    nc.sync.dma_start(out=t[:], in_=w[k])
                    pair.append(t)
                wh_sb.append(pair)
            bs_sb = []
            for li, b in enumerate(bs):
                t = cpool.tile([128, 2], F32, tag=f"b{li}")
                nc.sync.dma_start(out=t[:], in_=b[:, :])
                bs_sb.append(t)
            wf_sb = cpool.tile([128, 2], F32, tag="wf")
            nc.sync.dma_start(out=wf_sb[:], in_=wf[:, :])
            bf_sb = cpool.tile([1, 1], F32, tag="bf")
            nc.sync.dma_start(out=bf_sb[:], in_=bf[:, :])
            ones_sb = cpool.tile([128, 1], F32, tag="ones")
            nc.gpsimd.memset(ones_sb[:], 1.0)

            def bias_relu(h, ps, b_ap, use_act):
                if use_act:
                    nc.scalar.activation(h[:], ps[:], Relu, bias=b_ap)
                else:
                    nc.vector.tensor_scalar(
                        h[:], ps[:], b_ap, 0.0,
                        mybir.AluOpType.add, mybir.AluOpType.max)

            # process blocks in pairs so consecutive matmuls share the
            # same stationary (lhsT) operand
            for i0 in range(0, nblk, 2):
                blks = [i0] if i0 + 1 >= nblk else [i0, i0 + 1]
                nb = len(blks)
                cols = [slice(i * BLK, (i + 1) * BLK) for i in blks]
                if i0 == 0:
                    xta, xtb = xta0, xtb0
                else:
                    xta = xpool.tile([D_PAD, nb * BLK], F16, tag="xta")
                    nc.sync.dma_start(
                        out=xta[:], in_=xa[:, i0 * BLK:(i0 + nb) * BLK])
                    xtb = xpool.tile([D_PAD, nb * BLK], F16, tag="xtb")
                    nc.sync.dma_start(
                        out=xtb[:], in_=xb[:, i0 * BLK:(i0 + nb) * BLK])
                xtas = [xta[:, b * BLK:(b + 1) * BLK] for b in range(nb)]
                xtbs = [xtb[:, b * BLK:(b + 1) * BLK] for b in range(nb)]

                # layer 0: 27 -> 256, exact fp16x2 split
                # (x = x1 + x2/2048, w0 = w1 + w2/2048; the x2*w2 term is
                # ~2^-22 relative and dropped; fp16 products are exact in
                # the fp32 PSUM accumulator)
                psa = {}
                psb = {}
                for m in range(2):
                    ms = slice(m * 128, (m + 1) * 128)
                    for b in range(nb):
                        pa = ppool.tile([128, BLK], F32, tag="ps", name="pa")
                        nc.tensor.matmul(pa[:], lhsT=w0a_sb[:, ms],
                                         rhs=xtas[b], start=True, stop=True)
                        psa[(b, m)] = pa
                    for b in range(nb):
                        pb = ppool.tile([128, BLK], F32, tag="ps", name="pb")
                        nc.tensor.matmul(pb[:], lhsT=w0a_sb[:, ms],
                                         rhs=xtbs[b], start=True, stop=False)
                        nc.tensor.matmul(pb[:], lhsT=w0b_sb[:, ms],
                                         rhs=xtas[b], start=False, stop=True)
                        psb[(b, m)] = pb
                hcur = [[None, None] for _ in range(nb)]
                for m in range(2):
                    for b in range(nb):
                        u = hpool.tile([128, BLK], F32, tag="h", name="u")
                        nc.scalar.activation(u[:], psb[(b, m)][:], Identity,
                                             bias=bs_sb[0][:, m:m + 1],
                                             scale=1.0 / 2048.0)
                        t = hpool.tile([128, BLK], F32, tag="h", name="t")
                        nc.vector.tensor_tensor(t[:], psa[(b, m)][:], u[:],
                                                mybir.AluOpType.add)
                        h = hpool.tile([128, BLK], F32, tag="h")
                        if m == 0:
                            nc.scalar.activation(h[:], t[:], Relu)
                        else:
                            nc.vector.tensor_scalar(h[:], t[:], 0.0, None,
                                                    mybir.AluOpType.max)
                        hcur[b][m] = h

                # layers 1..3: 256 -> 256
                for li in range(3):
                    pss = {}
                    for m in range(2):
                        for k in range(2):
                            for b in range(nb):
                                if k == 0:
                                    ps = ppool.tile([128, BLK], F32, tag="ps")
                                    pss[(b, m)] = ps
                                nc.tensor.matmul(
                                    pss[(b, m)][:],
                                    lhsT=_mm(wh_sb[li][k][:, m * 128:(m + 1) * 128]),
                                    rhs=_mm(hcur[b][k][:]),
                                    start=(k == 0), stop=(k == 1))
                    hnew = [[None, None] for _ in range(nb)]
                    for m in range(2):
                        for b in range(nb):
                            h = hpool.tile([128, BLK], F32, tag="h")
                            bias_relu(h, pss[(b, m)], bs_sb[li + 1][:, m:m + 1],
                                      use_act=(m == 0))
                            hnew[b][m] = h
                    hcur = hnew

                # final: 256 -> 1.  Fold wf into the activations
                # elementwise (per-partition scale) and reduce the 128
                # partitions with a constant ones matmul: one PE pass per
                # block instead of two.
                psfs = []
                for b in range(nb):
                    t1 = hpool.tile([128, BLK], F32, tag="zf", name="t1")
                    nc.scalar.activation(t1[:], hcur[b][0][:],
                                         mybir.ActivationFunctionType.Copy,
                                         scale=wf_sb[:, 0:1])
                    t2 = hpool.tile([128, BLK], F32, tag="zf", name="t2")
                    nc.vector.tensor_scalar(t2[:], hcur[b][1][:],
                                            wf_sb[:, 1:2], None,
                                            mybir.AluOpType.mult)
                    z = hpool.tile([128, BLK], F32, tag="zf", name="z")
                    nc.vector.tensor_tensor(z[:], t1[:], t2[:],
                                            mybir.AluOpType.add)
                    psf = pfpool.tile([1, BLK], F32, tag="psf", name="psf")
                    nc.tensor.matmul(psf[:], lhsT=_mm(ones_sb[:]),
                                     rhs=_mm(z[:]), start=True, stop=True)
                    psfs.append(psf)
                for b in range(nb):
                    so = spool.tile([1, BLK], F32, tag="so")
                    nc.scalar.activation(so[:], psfs[b][:], Identity,
                                         bias=bf_sb[0:1, 0:1])
                    nc.sync.dma_start(out=sdf[:, cols[b]], in_=so[:])

    nc.compile()
    return nc


_NC_CACHE = {}


def _get_nc():
    if "nc" not in _NC_CACHE:
        _NC_CACHE["nc"] = _build_nc()
    return _NC_CACHE["nc"]


def _posenc_t(pos):
    """Feature-major positional encoding [27, N], fp32, matching the
    reference's fp32 elementwise ops."""
    n = pos.shape[0]
    posT = np.ascontiguousarray(pos.T.astype(np.float32, copy=False))  # [3, N]
    x = np.zeros((D_PAD, n), np.float32)
    x[0:3] = posT
    for i in range(FREQ_NUM):
        f = np.float32(float(2 ** i) * np.pi)
        fx = f * posT
        x[3 + 6 * i: 6 + 6 * i] = np.sin(fx)
        x[6 + 6 * i: 9 + 6 * i] = np.cos(fx)
    return x


def _split16(a):
    """Exact two-term fp16 split: a ~= a1 + a2/2048 to ~22 mantissa bits.
    a - fp32(a1) is exact (operands within 2x), *2048 is exact."""
    a1 = a.astype(np.float16)
    a2 = ((a - a1.astype(np.float32)) * np.float32(2048.0)).astype(np.float16)
    return np.ascontiguousarray(a1), np.ascontiguousarray(a2)


def _chunk_inputs(w0, b0, w1, b1, w2, b2, w3, b3, wf, bf):
    f32 = lambda a: np.ascontiguousarray(np.asarray(a), dtype=np.float32)
    w0p = np.zeros((D_PAD, HID), np.float32)
    w0p[:D_IN] = f32(w0)
    w0a, w0b = _split16(w0p)
    out = {"w0a": w0a, "w0b": w0b}
    for name, w in (("w1c", w1), ("w2c", w2), ("w3c", w3)):
        w = f32(w)
        out[name] = np.ascontiguousarray(np.stack([w[:128], w[128:]]))
    for name, b in (("b0c", b0), ("b1c", b1), ("b2c", b2), ("b3c", b3)):
        b = f32(b)
        out[name] = np.ascontiguousarray(b.reshape(2, 128).T)
    wf = f32(wf).reshape(-1)
    out["wfc"] = np.ascontiguousarray(wf.reshape(2, 128).T)
    out["bfc"] = f32(bf).reshape(1, 1)
    return out


def _sdf_device(x_t, weight_maps):
    """x_t: [27, N_VERTS] fp32 -> sdf [N_VERTS] fp32 via 8-core SPMD."""
    nc = _get_nc()
    in_maps = []
    for c in range(N_CORES):
        xc = np.zeros((D_PAD, PER_CORE_PAD), np.float32)
        xc[:, :PER_CORE] = x_t[:, c * PER_CORE:(c + 1) * PER_CORE]
        xca, xcb = _split16(xc)
        m = {"xa": xca, "xb": xcb}
        m.update(weight_maps)
        in_maps.append(m)
    res = run_bass_kernel_spmd(nc, in_maps, list(range(N_CORES)))
    sdf = np.empty(N_VERTS, np.float32)
    for c in range(N_CORES):
        sdf[c * PER_CORE:(c + 1) * PER_CORE] = \
            res.results[c]["sdf"][0, :PER_CORE]
    return sdf


def _march_tets(pos, tet_fx4, sdf):
    """Marching tetrahedra, numpy mirror of the jnp reference."""
    occ = sdf > 0
    occ_fx4 = occ[tet_fx4]                       # [F,4]
    occ_sum = occ_fx4.sum(-1)
    valid = (occ_sum > 0) & (occ_sum < 4)
    vt = tet_fx4[valid]                          # [V,4]
    occ_v = occ_fx4[valid]

    # Only crossing edges (occ[lo] != occ[hi]) ever appear in the
    # triangle tables, and the reference's compact ids are the ranks of
    # the crossing edges in (lo, hi)-sorted order — so uniquify just the
    # crossing instances; ranks and interp vertices come out identical.
    e = vt[:, _BASE_EDGES].reshape(-1, 2)        # [6V,2]
    e0, e1 = e[:, 0], e[:, 1]
    ce = occ[e0] != occ[e1]                      # crossing instances
    lo = np.minimum(e0[ce], e1[ce]).astype(np.int64)
    hi = np.maximum(e0[ce], e1[ce]).astype(np.int64)
    key = lo * N_VERTS + hi
    uniq_key, inv = np.unique(key, return_inverse=True)
    idx_flat = np.full(e.shape[0], -1, np.int32)
    idx_flat[ce] = inv.astype(np.int32)
    idx_map = idx_flat.reshape(-1, 6)

    ilo = uniq_key // N_VERTS
    ihi = uniq_key % N_VERTS
    s0 = sdf[ilo]
    s1 = sdf[ihi]
    denom = (s0 - s1)[:, None]
    verts = pos[ilo]
    verts = verts * (-s1)[:, None]
    np.add(verts, pos[ihi] * s0[:, None], out=verts)
    np.divide(verts, denom, out=verts)

    tetindex = (occ_v.astype(np.int32) *
                np.array([1, 2, 4, 8], np.int32)).sum(-1)
    ntri = _NUM_TRI[tetindex]
    tri = _TRI_TABLE[tetindex]
    m1 = ntri == 1
    m2 = ntri == 2
    f1 = np.take_along_axis(idx_map[m1], tri[m1][:, :3], axis=1).reshape(-1, 3)
    f2 = np.take_along_axis(idx_map[m2], tri[m2][:, :6], axis=1).reshape(-1, 3)
    faces = np.concatenate([f1, f2], axis=0).astype(np.int32)
    return verts.astype(np.float32), faces


def kernel(pos, tet_fx4, w0, b0, w1, b1, w2, b2, w3, b3, wf, bf):
    pos = np.ascontiguousarray(np.asarray(pos), dtype=np.float32)
    tet_fx4 = np.ascontiguousarray(np.asarray(tet_fx4), dtype=np.int32)
    x_t = _posenc_t(pos)
    wm = _chunk_inputs(w0, b0, w1, b1, w2, b2, w3, b3, wf, bf)
    sdf = _sdf_device(x_t, wm)
    return _march_tets(pos, tet_fx4, sdf)
